# revision 18
# baseline (speedup 1.0000x reference)
"""Trainium2 Bass kernel for nn_Cont_InfoNCE (pairwise max cross-correlation + CE loss).

Math: the reference's irfft(F1[i] * conj(F2[j]) / power) is the linear
cross-correlation of the centered rows at every lag, scaled by the positive
constant 1/(power*(T-1)).  max over lags therefore commutes with the scaling,
so dist[i,j] = max_l sum_t f1c[i,t] * f2c[j,t+l] / (1023*s1[i]*s2[j]).

The host centers the rows, folds 1/s2[j] into B, casts both operands to
fp8e4m3 and pre-builds the transposed B layout BT[t,c,j] = B[j,128c+t]; the
device computes the correlation at all lags as dense fp8 DoubleRow matmuls
on the tensor engine (fp32 PSUM accumulation), max-reduces over lags on the
vector engine, applies the 1/(1023*s1[i]) row scale, and does the row-wise
CE on device.  Sharding: rows of zis across the 8 cores (32 rows each), BT
replicated; each core emits a partial loss scalar and the host sums the 8
partials.

Host->device traffic per call is ~2.4 MB (fp8 operands) instead of the
9.2 MB of raw f32 inputs; the jitted shard_map executor is built once and
cached, so warm calls skip retracing/recompiling entirely.

Tiling (per core; Apad = padded fp8 rows of A):
  Apad[i]    = [0^255, A[i], 0^257]                       (32, 1536) fp8
  Tau[t,i,u] = Apad[i, u+t]          (Hankel gather via DMA from DRAM)
  for lam in 0..15, jt in 0..1, ic in 0..7:
    psum[j,ii,d'] += BT[:, 2dc:2dc+2, jtile].T @ Tau[:, ic, u0:u0+256]  (DoubleRow)
      over dc with u0 = 128*(2dc - lam + 9); pair halves are the two
      128-chunks of t, matching the production [P, ksub, free] convention.
  psum[j,ii,d'] equals C[i, j, l] at lag l = 128*lam - 897 - d', covering
  every lag in [-1024, 1023] exactly once (the l = -1024 slot is identically
  0, mirroring the reference's zero-overlap k=1024 slot).
"""

import sys

if "/opt/trn_rl_repo" not in sys.path:
    sys.path.insert(0, "/opt/trn_rl_repo")

from contextlib import ExitStack

import numpy as np

import concourse.bass as bass
import concourse.mybir as mybir
from concourse import bacc, tile
from concourse.masks import make_identity

F32 = mybir.dt.float32
BF16 = mybir.dt.bfloat16
FP8 = mybir.dt.float8e4
I32 = mybir.dt.int32
NP_FP8 = mybir.dt.np(FP8)
X = mybir.AxisListType.X
ALU = mybir.AluOpType
ACT = mybir.ActivationFunctionType
DROW = mybir.MatmulPerfMode.DoubleRow

M, T = 256, 1024
NCORES = 8
NLOC = M // NCORES  # 32 rows of zis per core
NIC = 4             # i-rows per i-chunk
NCHUNK = NLOC // NIC  # 8 i-chunks
TAU_U = 1408        # Hankel window extent: covers e0 in [-1, 8], +256 window
APAD = 1536         # 255 zeros + 1024 + 257 zeros


USE_COLLECTIVE = True
JBLK = M // NCORES  # 32 j-columns of BT shipped per core when gathering on-device


def build_nc():
    nc = bacc.Bacc("TRN2", target_bir_lowering=False, num_devices=NCORES)
    a8_in = nc.dram_tensor("a8", [NLOC, T], FP8, kind="ExternalInput")
    if USE_COLLECTIVE:
        # per-core j-slice of BT; all-gathered on device over the 8 cores
        bts = nc.dram_tensor("bts", [128, 8 * JBLK], FP8, kind="ExternalInput")
    else:
        btd = nc.dram_tensor("btd", [128, 8 * M], FP8, kind="ExternalInput")
    r1_d = nc.dram_tensor("r1", [NLOC, 1], F32, kind="ExternalInput")
    speeds_loc = nc.dram_tensor("speeds_loc", [NLOC, 1], I32, kind="ExternalInput")
    loss_part = nc.dram_tensor("loss_part", [1, 1], F32, kind="ExternalOutput")

    with tile.TileContext(nc) as tc, ExitStack() as ctx:
        consts = ctx.enter_context(tc.tile_pool(name="consts", bufs=1))
        prep = ctx.enter_context(tc.tile_pool(name="prep", bufs=2))
        taup = ctx.enter_context(tc.tile_pool(name="taup", bufs=3))
        dram = ctx.enter_context(tc.tile_pool(name="dram", bufs=1, space="DRAM"))
        ps_aux = ctx.enter_context(tc.tile_pool(name="ps_aux", bufs=2, space="PSUM"))
        ps_main = ctx.enter_context(tc.tile_pool(name="ps_main", bufs=3, space="PSUM"))

        # ---------------- zero-pad the A rows on device -----------------------
        # (ships T fp8 columns per row instead of APAD; tau's Hankel DMA needs
        # the padded layout to live in DRAM)
        asb = prep.tile([NLOC, APAD], FP8)
        nc.gpsimd.memset(asb, 0.0)
        nc.sync.dma_start(asb[:, 255:255 + T], a8_in[:, :])
        apad_d = dram.tile([NLOC, APAD], FP8)
        nc.sync.dma_start(apad_d[:, :], asb[:, :])

        # ---------------- constants / inputs ----------------
        ident_f32 = consts.tile([128, 128], F32)
        make_identity(nc, ident_f32)
        ones_col = consts.tile([NLOC, 1], F32)
        nc.gpsimd.memset(ones_col, 1.0)
        jidx_i = consts.tile([NLOC, M], I32)
        nc.gpsimd.iota(jidx_i, [[1, M]], base=0, channel_multiplier=0)
        jidx_f = consts.tile([NLOC, M], F32)
        nc.scalar.copy(jidx_f, jidx_i)
        sp_i = prep.tile([NLOC, 1], I32)
        nc.sync.dma_start(sp_i, speeds_loc[:, :])
        sp_f = prep.tile([NLOC, 1], F32)
        nc.scalar.copy(sp_f, sp_i)
        r1 = prep.tile([NLOC, 1], F32)
        nc.sync.dma_start(r1, r1_d[:, :])
        bt8 = consts.tile([128, 8, M], FP8)
        if USE_COLLECTIVE:
            # bounce the ExternalInput slice into a DRAM pool tile
            # (collectives may not touch kernel I/O tensors), all-gather the
            # 8 j-slices, then repack the canonical [t, c, j] SBUF layout.
            # TileContext tracks the bounce tiles and orders
            # dma -> collective -> repack automatically.
            bt_bounce = dram.tile([128, 8 * JBLK], FP8)
            bt_gather = dram.tile([NCORES * 128, 8 * JBLK], FP8)
            nc.gpsimd.dma_start(bt_bounce[:, :], bts[:, :])
            nc.gpsimd.collective_compute(
                "AllGather",
                mybir.AluOpType.bypass,
                replica_groups=[list(range(NCORES))],
                ins=[bt_bounce.opt()],
                outs=[bt_gather.opt()],
            )
            for r in range(NCORES):
                nc.sync.dma_start(
                    bt8[:, :, JBLK * r:JBLK * (r + 1)],
                    bt_gather[128 * r:128 * (r + 1), :].rearrange(
                        "p (c j) -> p c j", c=8
                    ),
                )
        else:
            nc.sync.dma_start(bt8[:, :, :], btd[:, :].rearrange("p (c j) -> p c j", c=8))

        # ---------------- main correlation loop ------------------------------
        cmax_p = [
            consts.tile([128, 16, NLOC], F32, tag=f"cmax_{jt}", name=f"cmax_{jt}")
            for jt in range(2)
        ]
        for ic in range(NCHUNK):
            tau = taup.tile([128, NIC, TAU_U], FP8, tag="tau")
            src = apad_d[NIC * ic:NIC * (ic + 1), 0:TAU_U]
            v = src.unsqueeze(0).broadcast_to((128, NIC, TAU_U))
            lst = v.ap
            lst[0] = [1, 128]  # Hankel: dest partition t reads Apad at +t elements
            v.ap = lst
            nc.sync.dma_start(tau[:, :, :], v)
            for jt in range(2):
                for lp in range(8):  # lambda pairs -> one 2-bank psum tile
                    ps = ps_main.tile([128, 2, NIC, 128], F32, tag="grp")
                    for q in range(2):
                        lam = 2 * lp + q
                        # valid double-chunks: e0 = 2dc - lam + 8 in [-1, 8]
                        dcs = [dc for dc in range(4) if -1 <= 2 * dc - lam + 8 <= 8]
                        for k, dc in enumerate(dcs):
                            u0 = 128 * (2 * dc - lam + 9)
                            rhs = tau[:, :, u0:u0 + 256].rearrange(
                                "p r (i d) -> p i r d", i=2
                            )
                            nc.tensor.matmul(
                                ps[:, q],
                                lhsT=bt8[:, 2 * dc:2 * dc + 2, jt * 128:(jt + 1) * 128],
                                rhs=rhs,
                                perf_mode=DROW,
                                start=(k == 0),
                                stop=(k == len(dcs) - 1),
                            )
                    nc.vector.reduce_max(
                        cmax_p[jt][:, 2 * lp:2 * lp + 2, NIC * ic:NIC * (ic + 1)],
                        ps[:, :, :, :],
                        axis=X,
                    )

        # ---------------- reduce over lag groups + transpose to (i, j) --------
        dist_t = prep.tile([NLOC, M], F32)
        for jt in range(2):
            cm2 = prep.tile([128, NLOC], F32, tag="cm2")
            nc.vector.reduce_max(cm2, cmax_p[jt].rearrange("p l i -> p i l"), axis=X)
            ps_d = ps_aux.tile([NLOC, 128], F32, tag="aux")
            nc.tensor.transpose(ps_d, cm2, ident_f32)
            nc.vector.tensor_scalar(dist_t[:, jt * 128:(jt + 1) * 128], ps_d, r1, None, op0=ALU.mult)

        # ---------------- cross-entropy (sum over local rows) -----------------
        mrow = prep.tile([NLOC, 1], F32)
        nc.vector.reduce_max(mrow, dist_t, axis=X)
        negm = prep.tile([NLOC, 1], F32)
        nc.vector.tensor_scalar_mul(negm, mrow, -1.0)
        expj = prep.tile([NLOC, M], F32)
        sumexp = prep.tile([NLOC, 1], F32)
        nc.scalar.activation(expj, dist_t, ACT.Exp, bias=negm, accum_out=sumexp)
        lse = prep.tile([NLOC, 1], F32)
        nc.scalar.activation(lse, sumexp, ACT.Ln)
        onehot = prep.tile([NLOC, M], F32)
        nc.vector.tensor_scalar(onehot, jidx_f, sp_f, None, op0=ALU.is_equal)
        junk_p = prep.tile([NLOC, M], F32)
        picked = prep.tile([NLOC, 1], F32)
        nc.vector.scalar_tensor_tensor(
            junk_p, in0=dist_t, scalar=1.0, in1=onehot, op0=ALU.mult, op1=ALU.mult, accum_out=picked
        )
        term = prep.tile([NLOC, 1], F32)
        nc.vector.tensor_add(term, lse, mrow)
        term2 = prep.tile([NLOC, 1], F32)
        nc.vector.tensor_sub(term2, term, picked)
        ps_l = ps_aux.tile([1, 1], F32, tag="aux")
        nc.tensor.matmul(ps_l, lhsT=term2, rhs=ones_col, start=True, stop=True)
        lsb = prep.tile([1, 1], F32)
        nc.vector.tensor_copy(lsb, ps_l)
        nc.sync.dma_start(loss_part[:, :], lsb)

    nc.finalize()
    return nc


_RUNNER = None
LAST_RESULT = None


def _make_runner():
    """Build the Bass module and a persistently-cached jitted executor.

    run_bass_kernel_spmd rebuilds its jit closure on every call, so each
    call re-traces, re-runs BIR verify/optimise and XLA compile (~0.5 s)
    and re-fetches the sharded output once per core.  Here the
    jax.jit(shard_map(...)) wrapper is built exactly once; warm calls hit
    the pjit C++ fast path and do a single host<->device round trip.
    """
    import jax
    from jax.experimental.shard_map import shard_map
    from jax.sharding import Mesh, PartitionSpec

    from concourse import bass2jax

    nc = build_nc()
    bass2jax.install_neuronx_cc_hook()
    assert nc.dbg_addr is None or not nc.dbg_callbacks
    partition_name = nc.partition_id_tensor.name if nc.partition_id_tensor else None

    in_names, out_names, out_avals = [], [], []
    for alloc in nc.m.functions[0].allocations:
        if not isinstance(alloc, mybir.MemoryLocationSet):
            continue
        name = alloc.memorylocations[0].name
        if alloc.kind == "ExternalInput":
            if name != partition_name:
                in_names.append(name)
        elif alloc.kind == "ExternalOutput":
            out_names.append(name)
            out_avals.append(
                jax.core.ShapedArray(tuple(alloc.tensor_shape), mybir.dt.np(alloc.dtype))
            )
    n_params = len(in_names)
    n_outs = len(out_avals)
    all_in_names = tuple(in_names + out_names + ([partition_name] if partition_name else []))
    donate = tuple(range(n_params, n_params + n_outs))

    def _body(*args):
        operands = list(args)
        if partition_name is not None:
            operands.append(bass2jax.partition_id_tensor())
        outs = bass2jax._bass_exec_p.bind(
            *operands,
            out_avals=tuple(out_avals),
            in_names=all_in_names,
            out_names=tuple(out_names),
            lowering_input_output_aliases=(),
            sim_require_finite=True,
            sim_require_nnan=True,
            nc=nc,
        )
        return tuple(outs)

    devices = jax.devices()[:NCORES]
    mesh = Mesh(np.asarray(devices), ("core",))
    in_specs = (PartitionSpec("core"),) * (n_params + n_outs)
    out_specs = (PartitionSpec("core"),) * n_outs
    sharded = jax.jit(
        shard_map(_body, mesh=mesh, in_specs=in_specs, out_specs=out_specs, check_rep=False),
        donate_argnums=donate,
        keep_unused=True,
    )
    zero_shapes = [
        ((NCORES * a.shape[0],) + tuple(a.shape[1:]), a.dtype) for a in out_avals
    ]
    in_sharding = jax.sharding.NamedSharding(mesh, PartitionSpec("core"))

    def put(arr):
        """Async host->device transfer with the row-block sharding."""
        return jax.device_put(arr, in_sharding)

    def call(concat_inputs):
        """concat_inputs: dict name -> global (NCORES*rows, ...) array."""
        ins = [concat_inputs[name] for name in in_names]
        zeros = [np.zeros(s, d) for s, d in zero_shapes]
        out_arrs = sharded(*ins, *zeros)
        return [np.asarray(o) for o in out_arrs]

    return call, put


_F16_TO_FP8 = None


def _to_fp8(x):
    """Fast f32 -> fp8e4m3 via f16 + 64K lookup (ml_dtypes scalar cast is slow)."""
    global _F16_TO_FP8
    if _F16_TO_FP8 is None:
        all16 = np.arange(65536, dtype=np.uint16).view(np.float16)
        with np.errstate(all="ignore"):
            _F16_TO_FP8 = all16.astype(np.float32).astype(NP_FP8).view(np.uint8)
    idx = x.astype(np.float16).view(np.uint16)
    return _F16_TO_FP8[idx].view(NP_FP8)


def _center(z):
    zc = z - z.mean(axis=-1, keepdims=True)
    ss = np.einsum("ij,ij->i", zc, zc)
    s = np.sqrt(ss / (T - 1))
    return zc, np.where(s == 0.0, 1.0, s)


def _prep_a(zis):
    """a8 = fp8(centered zis); r1 = 1/((T-1)*s1)."""
    f1c, s1 = _center(np.asarray(zis, dtype=np.float32))
    r1 = (1.0 / ((T - 1) * s1)).astype(np.float32).reshape(M, 1)
    return _to_fp8(f1c), r1


def _prep_bts(zjs):
    """Per-core j-slices of BT[t,c,j] = (f2c/s2)[j, 128c+t], fp8.

    Returns the (NCORES*128, 8*JBLK) stack fed to the on-device AllGather:
    row block r holds BT[:, :, JBLK*r : JBLK*(r+1)] as [t, (c, j')].
    """
    f2c, s2 = _center(np.asarray(zjs, dtype=np.float32))
    b8 = _to_fp8(f2c * (1.0 / s2)[:, None])  # (M, T)
    # b8[32r + j', 128c + t] -> bts[(r, t), (c, j')] in one copy
    return np.ascontiguousarray(
        b8.reshape(NCORES, JBLK, 8, 128).transpose(0, 3, 2, 1)
    ).reshape(NCORES * 128, 8 * JBLK)


def run(zis, zjs, speeds, trace=False):
    global _RUNNER
    if _RUNNER is None:
        _RUNNER = _make_runner()
    call, put = _RUNNER
    # start each transfer as soon as its tensor is ready so the tunnel
    # transfer overlaps the remaining host-side prep
    a8, r1 = _prep_a(zis)
    concat_inputs = {"a8": put(a8)}
    if USE_COLLECTIVE:
        concat_inputs["bts"] = put(_prep_bts(zjs))
    else:
        f2c, s2 = _center(np.asarray(zjs, dtype=np.float32))
        b8 = _to_fp8(f2c * (1.0 / s2)[:, None])
        bt = np.ascontiguousarray(
            b8.reshape(M, 8, 128).transpose(2, 1, 0)
        ).reshape(128, 8 * M)
        concat_inputs["btd"] = put(np.tile(bt, (NCORES, 1)))
    sp = np.ascontiguousarray(np.asarray(speeds).astype(np.int32).reshape(M, 1))
    concat_inputs["r1"] = put(r1)
    concat_inputs["speeds_loc"] = put(sp)
    outs = call(concat_inputs)
    loss_parts = outs[0].reshape(NCORES)
    return np.float32(float(loss_parts.sum()))


def kernel(zis, zjs, speeds):
    return run(zis, zjs, speeds, trace=False)


# revision 19
# speedup vs baseline: 1.0974x; 1.0974x over previous
"""Trainium2 Bass kernel for nn_Cont_InfoNCE (pairwise max cross-correlation + CE loss).

Math: the reference's irfft(F1[i] * conj(F2[j]) / power) is the linear
cross-correlation of the centered rows at every lag, scaled by the positive
constant 1/(power*(T-1)).  max over lags therefore commutes with the scaling,
so dist[i,j] = max_l sum_t f1c[i,t] * f2c[j,t+l] / (1023*s1[i]*s2[j]).

The host centers the rows, folds 1/s2[j] into B, casts both operands to
fp8e4m3 and pre-builds the transposed B layout BT[t,c,j] = B[j,128c+t]; the
device computes the correlation at all lags as dense fp8 DoubleRow matmuls
on the tensor engine (fp32 PSUM accumulation), max-reduces over lags on the
vector engine, applies the 1/(1023*s1[i]) row scale, and does the row-wise
CE on device.  Sharding: rows of zis across the 8 cores (32 rows each), BT
replicated; each core emits a partial loss scalar and the host sums the 8
partials.

Host->device traffic per call is ~0.53 MB: both fp8 operands are shipped
fully sharded (32 KB of A rows + 32 KB of BT j-columns per core) and BT is
all-gathered across the 8 cores on device, A is zero-padded on device, and
transfers start asynchronously while the rest of the host prep runs.  The
jitted shard_map executor is built once and cached, so warm calls skip
retracing/recompiling entirely; each warm call costs one axon round trip
(~50 ms floor in this environment) plus ~3 ms of host prep.

Tiling (per core; Apad = padded fp8 rows of A):
  Apad[i]    = [0^255, A[i], 0^257]                       (32, 1536) fp8
  Tau[t,i,u] = Apad[i, u+t]          (Hankel gather via DMA from DRAM)
  for lam in 0..15, jt in 0..1, ic in 0..7:
    psum[j,ii,d'] += BT[:, 2dc:2dc+2, jtile].T @ Tau[:, ic, u0:u0+256]  (DoubleRow)
      over dc with u0 = 128*(2dc - lam + 9); pair halves are the two
      128-chunks of t, matching the production [P, ksub, free] convention.
  psum[j,ii,d'] equals C[i, j, l] at lag l = 128*lam - 897 - d', covering
  every lag in [-1024, 1023] exactly once (the l = -1024 slot is identically
  0, mirroring the reference's zero-overlap k=1024 slot).
"""

import sys

if "/opt/trn_rl_repo" not in sys.path:
    sys.path.insert(0, "/opt/trn_rl_repo")

from contextlib import ExitStack

import numpy as np

import concourse.bass as bass
import concourse.mybir as mybir
from concourse import bacc, tile
from concourse.masks import make_identity

F32 = mybir.dt.float32
BF16 = mybir.dt.bfloat16
FP8 = mybir.dt.float8e4
I32 = mybir.dt.int32
NP_FP8 = mybir.dt.np(FP8)
X = mybir.AxisListType.X
ALU = mybir.AluOpType
ACT = mybir.ActivationFunctionType
DROW = mybir.MatmulPerfMode.DoubleRow

M, T = 256, 1024
NCORES = 8
NLOC = M // NCORES  # 32 rows of zis per core
NIC = 4             # i-rows per i-chunk
NCHUNK = NLOC // NIC  # 8 i-chunks
TAU_U = 1408        # Hankel window extent: covers e0 in [-1, 8], +256 window
APAD = 1536         # 255 zeros + 1024 + 257 zeros


USE_COLLECTIVE = True
JBLK = M // NCORES  # 32 j-columns of BT shipped per core when gathering on-device


def build_nc():
    nc = bacc.Bacc("TRN2", target_bir_lowering=False, num_devices=NCORES)
    a8_in = nc.dram_tensor("a8", [NLOC, T], FP8, kind="ExternalInput")
    if USE_COLLECTIVE:
        # per-core j-slice of BT; all-gathered on device over the 8 cores
        bts = nc.dram_tensor("bts", [128, 8 * JBLK], FP8, kind="ExternalInput")
    else:
        btd = nc.dram_tensor("btd", [128, 8 * M], FP8, kind="ExternalInput")
    r1_d = nc.dram_tensor("r1", [NLOC, 1], F32, kind="ExternalInput")
    speeds_loc = nc.dram_tensor("speeds_loc", [NLOC, 1], I32, kind="ExternalInput")
    loss_part = nc.dram_tensor("loss_part", [1, 1], F32, kind="ExternalOutput")

    with tile.TileContext(nc) as tc, ExitStack() as ctx:
        consts = ctx.enter_context(tc.tile_pool(name="consts", bufs=1))
        prep = ctx.enter_context(tc.tile_pool(name="prep", bufs=2))
        taup = ctx.enter_context(tc.tile_pool(name="taup", bufs=3))
        dram = ctx.enter_context(tc.tile_pool(name="dram", bufs=1, space="DRAM"))
        ps_aux = ctx.enter_context(tc.tile_pool(name="ps_aux", bufs=2, space="PSUM"))
        ps_main = ctx.enter_context(tc.tile_pool(name="ps_main", bufs=3, space="PSUM"))

        # ---------------- zero-pad the A rows on device -----------------------
        # (ships T fp8 columns per row instead of APAD; tau's Hankel DMA needs
        # the padded layout to live in DRAM)
        asb = prep.tile([NLOC, APAD], FP8)
        nc.gpsimd.memset(asb, 0.0)
        nc.sync.dma_start(asb[:, 255:255 + T], a8_in[:, :])
        apad_d = dram.tile([NLOC, APAD], FP8)
        nc.sync.dma_start(apad_d[:, :], asb[:, :])

        # ---------------- constants / inputs ----------------
        ident_f32 = consts.tile([128, 128], F32)
        make_identity(nc, ident_f32)
        ones_col = consts.tile([NLOC, 1], F32)
        nc.gpsimd.memset(ones_col, 1.0)
        jidx_i = consts.tile([NLOC, M], I32)
        nc.gpsimd.iota(jidx_i, [[1, M]], base=0, channel_multiplier=0)
        jidx_f = consts.tile([NLOC, M], F32)
        nc.scalar.copy(jidx_f, jidx_i)
        sp_i = prep.tile([NLOC, 1], I32)
        nc.sync.dma_start(sp_i, speeds_loc[:, :])
        sp_f = prep.tile([NLOC, 1], F32)
        nc.scalar.copy(sp_f, sp_i)
        r1 = prep.tile([NLOC, 1], F32)
        nc.sync.dma_start(r1, r1_d[:, :])
        bt8 = consts.tile([128, 8, M], FP8)
        if USE_COLLECTIVE:
            # bounce the ExternalInput slice into a DRAM pool tile
            # (collectives may not touch kernel I/O tensors), all-gather the
            # 8 j-slices, then repack the canonical [t, c, j] SBUF layout.
            # TileContext tracks the bounce tiles and orders
            # dma -> collective -> repack automatically.
            bt_bounce = dram.tile([128, 8 * JBLK], FP8)
            bt_gather = dram.tile([NCORES * 128, 8 * JBLK], FP8)
            nc.gpsimd.dma_start(bt_bounce[:, :], bts[:, :])
            nc.gpsimd.collective_compute(
                "AllGather",
                mybir.AluOpType.bypass,
                replica_groups=[list(range(NCORES))],
                ins=[bt_bounce.opt()],
                outs=[bt_gather.opt()],
            )
            for r in range(NCORES):
                nc.sync.dma_start(
                    bt8[:, :, JBLK * r:JBLK * (r + 1)],
                    bt_gather[128 * r:128 * (r + 1), :].rearrange(
                        "p (c j) -> p c j", c=8
                    ),
                )
        else:
            nc.sync.dma_start(bt8[:, :, :], btd[:, :].rearrange("p (c j) -> p c j", c=8))

        # ---------------- main correlation loop ------------------------------
        cmax_p = [
            consts.tile([128, 16, NLOC], F32, tag=f"cmax_{jt}", name=f"cmax_{jt}")
            for jt in range(2)
        ]
        for ic in range(NCHUNK):
            tau = taup.tile([128, NIC, TAU_U], FP8, tag="tau")
            src = apad_d[NIC * ic:NIC * (ic + 1), 0:TAU_U]
            v = src.unsqueeze(0).broadcast_to((128, NIC, TAU_U))
            lst = v.ap
            lst[0] = [1, 128]  # Hankel: dest partition t reads Apad at +t elements
            v.ap = lst
            nc.sync.dma_start(tau[:, :, :], v)
            for jt in range(2):
                for lp in range(8):  # lambda pairs -> one 2-bank psum tile
                    ps = ps_main.tile([128, 2, NIC, 128], F32, tag="grp")
                    for q in range(2):
                        lam = 2 * lp + q
                        # valid double-chunks: e0 = 2dc - lam + 8 in [-1, 8]
                        dcs = [dc for dc in range(4) if -1 <= 2 * dc - lam + 8 <= 8]
                        for k, dc in enumerate(dcs):
                            u0 = 128 * (2 * dc - lam + 9)
                            rhs = tau[:, :, u0:u0 + 256].rearrange(
                                "p r (i d) -> p i r d", i=2
                            )
                            nc.tensor.matmul(
                                ps[:, q],
                                lhsT=bt8[:, 2 * dc:2 * dc + 2, jt * 128:(jt + 1) * 128],
                                rhs=rhs,
                                perf_mode=DROW,
                                start=(k == 0),
                                stop=(k == len(dcs) - 1),
                            )
                    nc.vector.reduce_max(
                        cmax_p[jt][:, 2 * lp:2 * lp + 2, NIC * ic:NIC * (ic + 1)],
                        ps[:, :, :, :],
                        axis=X,
                    )

        # ---------------- reduce over lag groups + transpose to (i, j) --------
        dist_t = prep.tile([NLOC, M], F32)
        for jt in range(2):
            cm2 = prep.tile([128, NLOC], F32, tag="cm2")
            nc.vector.reduce_max(cm2, cmax_p[jt].rearrange("p l i -> p i l"), axis=X)
            ps_d = ps_aux.tile([NLOC, 128], F32, tag="aux")
            nc.tensor.transpose(ps_d, cm2, ident_f32)
            nc.vector.tensor_scalar(dist_t[:, jt * 128:(jt + 1) * 128], ps_d, r1, None, op0=ALU.mult)

        # ---------------- cross-entropy (sum over local rows) -----------------
        mrow = prep.tile([NLOC, 1], F32)
        nc.vector.reduce_max(mrow, dist_t, axis=X)
        negm = prep.tile([NLOC, 1], F32)
        nc.vector.tensor_scalar_mul(negm, mrow, -1.0)
        expj = prep.tile([NLOC, M], F32)
        sumexp = prep.tile([NLOC, 1], F32)
        nc.scalar.activation(expj, dist_t, ACT.Exp, bias=negm, accum_out=sumexp)
        lse = prep.tile([NLOC, 1], F32)
        nc.scalar.activation(lse, sumexp, ACT.Ln)
        onehot = prep.tile([NLOC, M], F32)
        nc.vector.tensor_scalar(onehot, jidx_f, sp_f, None, op0=ALU.is_equal)
        junk_p = prep.tile([NLOC, M], F32)
        picked = prep.tile([NLOC, 1], F32)
        nc.vector.scalar_tensor_tensor(
            junk_p, in0=dist_t, scalar=1.0, in1=onehot, op0=ALU.mult, op1=ALU.mult, accum_out=picked
        )
        term = prep.tile([NLOC, 1], F32)
        nc.vector.tensor_add(term, lse, mrow)
        term2 = prep.tile([NLOC, 1], F32)
        nc.vector.tensor_sub(term2, term, picked)
        ps_l = ps_aux.tile([1, 1], F32, tag="aux")
        nc.tensor.matmul(ps_l, lhsT=term2, rhs=ones_col, start=True, stop=True)
        lsb = prep.tile([1, 1], F32)
        nc.vector.tensor_copy(lsb, ps_l)
        nc.sync.dma_start(loss_part[:, :], lsb)

    nc.finalize()
    return nc


_RUNNER = None
LAST_RESULT = None


def _make_runner():
    """Build the Bass module and a persistently-cached jitted executor.

    run_bass_kernel_spmd rebuilds its jit closure on every call, so each
    call re-traces, re-runs BIR verify/optimise and XLA compile (~0.5 s)
    and re-fetches the sharded output once per core.  Here the
    jax.jit(shard_map(...)) wrapper is built exactly once; warm calls hit
    the pjit C++ fast path and do a single host<->device round trip.
    """
    import jax
    from jax.experimental.shard_map import shard_map
    from jax.sharding import Mesh, PartitionSpec

    from concourse import bass2jax

    nc = build_nc()
    bass2jax.install_neuronx_cc_hook()
    assert nc.dbg_addr is None or not nc.dbg_callbacks
    partition_name = nc.partition_id_tensor.name if nc.partition_id_tensor else None

    in_names, out_names, out_avals = [], [], []
    for alloc in nc.m.functions[0].allocations:
        if not isinstance(alloc, mybir.MemoryLocationSet):
            continue
        name = alloc.memorylocations[0].name
        if alloc.kind == "ExternalInput":
            if name != partition_name:
                in_names.append(name)
        elif alloc.kind == "ExternalOutput":
            out_names.append(name)
            out_avals.append(
                jax.core.ShapedArray(tuple(alloc.tensor_shape), mybir.dt.np(alloc.dtype))
            )
    n_params = len(in_names)
    n_outs = len(out_avals)
    all_in_names = tuple(in_names + out_names + ([partition_name] if partition_name else []))
    donate = tuple(range(n_params, n_params + n_outs))

    def _body(*args):
        operands = list(args)
        if partition_name is not None:
            operands.append(bass2jax.partition_id_tensor())
        outs = bass2jax._bass_exec_p.bind(
            *operands,
            out_avals=tuple(out_avals),
            in_names=all_in_names,
            out_names=tuple(out_names),
            lowering_input_output_aliases=(),
            sim_require_finite=True,
            sim_require_nnan=True,
            nc=nc,
        )
        return tuple(outs)

    devices = jax.devices()[:NCORES]
    mesh = Mesh(np.asarray(devices), ("core",))
    in_specs = (PartitionSpec("core"),) * (n_params + n_outs)
    out_specs = (PartitionSpec("core"),) * n_outs
    sharded = jax.jit(
        shard_map(_body, mesh=mesh, in_specs=in_specs, out_specs=out_specs, check_rep=False),
        donate_argnums=donate,
        keep_unused=True,
    )
    zero_shapes = [
        ((NCORES * a.shape[0],) + tuple(a.shape[1:]), a.dtype) for a in out_avals
    ]
    in_sharding = jax.sharding.NamedSharding(mesh, PartitionSpec("core"))

    def put(arr):
        """Async host->device transfer with the row-block sharding."""
        return jax.device_put(arr, in_sharding)

    def call(concat_inputs):
        """concat_inputs: dict name -> global (NCORES*rows, ...) array."""
        ins = [concat_inputs[name] for name in in_names]
        zeros = [np.zeros(s, d) for s, d in zero_shapes]
        out_arrs = sharded(*ins, *zeros)
        return [np.asarray(o) for o in out_arrs]

    return call, put


_F16_TO_FP8 = None


def _to_fp8(x):
    """Fast f32 -> fp8e4m3 via f16 + 64K lookup (ml_dtypes scalar cast is slow)."""
    global _F16_TO_FP8
    if _F16_TO_FP8 is None:
        all16 = np.arange(65536, dtype=np.uint16).view(np.float16)
        with np.errstate(all="ignore"):
            _F16_TO_FP8 = all16.astype(np.float32).astype(NP_FP8).view(np.uint8)
    idx = x.astype(np.float16).view(np.uint16)
    return _F16_TO_FP8[idx].view(NP_FP8)


def _center(z):
    zc = z - z.mean(axis=-1, keepdims=True)
    ss = np.einsum("ij,ij->i", zc, zc)
    s = np.sqrt(ss / (T - 1))
    return zc, np.where(s == 0.0, 1.0, s)


def _prep_a(zis):
    """a8 = fp8(centered zis); r1 = 1/((T-1)*s1)."""
    f1c, s1 = _center(np.asarray(zis, dtype=np.float32))
    r1 = (1.0 / ((T - 1) * s1)).astype(np.float32).reshape(M, 1)
    return _to_fp8(f1c), r1


def _prep_bts(zjs):
    """Per-core j-slices of BT[t,c,j] = (f2c/s2)[j, 128c+t], fp8.

    Returns the (NCORES*128, 8*JBLK) stack fed to the on-device AllGather:
    row block r holds BT[:, :, JBLK*r : JBLK*(r+1)] as [t, (c, j')].
    """
    f2c, s2 = _center(np.asarray(zjs, dtype=np.float32))
    b8 = _to_fp8(f2c * (1.0 / s2)[:, None])  # (M, T)
    # b8[32r + j', 128c + t] -> bts[(r, t), (c, j')] in one copy
    return np.ascontiguousarray(
        b8.reshape(NCORES, JBLK, 8, 128).transpose(0, 3, 2, 1)
    ).reshape(NCORES * 128, 8 * JBLK)


def run(zis, zjs, speeds, trace=False):
    global _RUNNER
    if _RUNNER is None:
        _RUNNER = _make_runner()
    call, put = _RUNNER
    # start each transfer as soon as its tensor is ready so the tunnel
    # transfer overlaps the remaining host-side prep
    a8, r1 = _prep_a(zis)
    concat_inputs = {"a8": put(a8)}
    if USE_COLLECTIVE:
        concat_inputs["bts"] = put(_prep_bts(zjs))
    else:
        f2c, s2 = _center(np.asarray(zjs, dtype=np.float32))
        b8 = _to_fp8(f2c * (1.0 / s2)[:, None])
        bt = np.ascontiguousarray(
            b8.reshape(M, 8, 128).transpose(2, 1, 0)
        ).reshape(128, 8 * M)
        concat_inputs["btd"] = put(np.tile(bt, (NCORES, 1)))
    sp = np.ascontiguousarray(np.asarray(speeds).astype(np.int32).reshape(M, 1))
    concat_inputs["r1"] = put(r1)
    concat_inputs["speeds_loc"] = put(sp)
    outs = call(concat_inputs)
    loss_parts = outs[0].reshape(NCORES)
    return np.float32(float(loss_parts.sum()))


def kernel(zis, zjs, speeds):
    return run(zis, zjs, speeds, trace=False)
